# revision 22
# baseline (speedup 1.0000x reference)
"""Causal self-attention (B=2, T=2048, D=1024, H=16) on 8 trn2 NeuronCores.

Sharding: core c handles batch b=c//4 and head group g=c%4 (4 heads each).
Each core computes q/k/v projections for its heads, causal attention, and a
partial output projection against its slice of Wo; the host sums the 4
partials per batch.
"""

import os
import sys
import types

sys.path.insert(0, "/opt/trn_rl_repo")

import numpy as np
import orjson

import concourse.bass as bass
import concourse.mybir as mybir
import concourse.tile as tile
from concourse.masks import make_identity

f32 = mybir.dt.float32
bf16 = mybir.dt.bfloat16

T = 2048
C = 1024
NH = 4          # heads per core
DH = 64
QB = 512        # query block
NQB = T // QB   # 4
KT = 128        # key tile
NKT = T // KT   # 16
CCH = C // 128  # 8 contraction chunks


def _install_ntff_hook():
    """Provide antenv.axon_hooks (absent in this image) so trace=True /
    BASS_TRACE=1 profiling works under axon instead of crashing."""
    try:
        import antenv.axon_hooks  # noqa: F401
        return
    except ImportError:
        pass
    try:
        mod = types.ModuleType("antenv.axon_hooks")
        _h = [None]
        mod.set_axon_ntff_profile_hook = lambda h: _h.__setitem__(0, h)
        mod.get_axon_ntff_profile_hook = lambda: _h[0]
        sys.modules["antenv.axon_hooks"] = mod
        import antenv

        antenv.axon_hooks = mod
        from trn_agent_boot.trn_boot import _ntff_profile_via_ctypes

        so = "/opt/axon/libaxon_pjrt.so"
        if os.path.exists(so):
            hook = _ntff_profile_via_ctypes(so)
            if hook is not None:
                mod.set_axon_ntff_profile_hook(hook)
    except Exception:
        pass


_install_ntff_hook()



def _split_multi_waits(bir: dict) -> dict:
    """The walrus build here rejects instructions with >1 semaphore wait.
    Hoist extra waits onto EventSemaphore insts inserted just before the
    offending instruction on the same engine (semantically equivalent)."""
    for fn in bir.get("functions", []):
        for bb in fn.get("blocks", []):
            out = []
            changed = False
            for inst in bb.get("instructions", []):
                si = inst.get("sync_info")
                waits = si.get("on_wait") if si else None
                if waits and len(waits) > 1:
                    changed = True
                    for i, w in enumerate(waits[:-1]):
                        out.append(
                            {
                                "debug": inst.get("debug", 0),
                                "engine": inst["engine"],
                                "ins": [],
                                "name": f"{inst.get('name', 'I')}-hw{i}",
                                "opcode": "EventSemaphore",
                                "outs": [],
                                "sync_info": {"on_update": [], "on_wait": [w]},
                            }
                        )
                    si["on_wait"] = [waits[-1]]
                out.append(inst)
            if changed:
                bb["instructions"] = out
    return bir


class PatchedBass(bass.Bass):
    def to_json_bytes(self) -> bytes:
        raw = super().to_json_bytes()
        return orjson.dumps(_split_multi_waits(orjson.loads(raw)))


def build_nc():
    nc = PatchedBass(name="causal_attn")
    xt = nc.dram_tensor("xt", [C, T], bf16, kind="ExternalInput")
    wq = nc.dram_tensor("wq", [C, NH * DH], bf16, kind="ExternalInput")
    wk = nc.dram_tensor("wk", [C, NH * DH], bf16, kind="ExternalInput")
    wv = nc.dram_tensor("wv", [C, NH * DH], bf16, kind="ExternalInput")
    wo = nc.dram_tensor("wo", [NH * DH, C], bf16, kind="ExternalInput")
    out = nc.dram_tensor("out", [T, C], f32, kind="ExternalOutput")

    from contextlib import ExitStack

    with tile.TileContext(nc) as tc, ExitStack() as ctx:
        cp = ctx.enter_context(tc.tile_pool(name="const", bufs=1))
        # persistent SBUF tensors
        xt_c = [cp.tile([128, T], bf16, name=f"xt_c{kc}") for kc in range(CCH)]
        wq_sb = cp.tile([128, CCH, NH * DH], bf16)
        wk_sb = cp.tile([128, CCH, NH * DH], bf16)
        wv_sb = cp.tile([128, CCH, NH * DH], bf16)
        wo_sb = cp.tile([128, 2, C], bf16)
        qt_sb = cp.tile([128, 2, T], bf16)
        kt_sb = cp.tile([128, 2, T], bf16)
        vv_sb = cp.tile([128, NKT, NH, DH + 1], bf16)
        otu_sb = cp.tile([128, 2, T], f32)
        yt_sb = cp.tile([128, 2, T], bf16)
        masks_sb = cp.tile([128, NQB, QB], bf16)
        ident_sb = cp.tile([128, 128], f32)
        rs_sb = cp.tile([128, T], f32)
        rst_sb = cp.tile([128, NKT, NH], f32)
        rrt_sb = cp.tile([128, NKT, NH], f32)
        rr4_sb = cp.tile([NH, T], f32)
        rb_sb = cp.tile([128, 2, T], f32)

        # weights on the gpsimd queue, xt per-chunk on sync so projection
        # matmuls can start as soon as each chunk lands
        nc.sync.dma_start(wq_sb[:], wq.rearrange("(kc p) m -> p kc m", p=128))
        nc.gpsimd.dma_start(wk_sb[:], wk.rearrange("(kc p) m -> p kc m", p=128))
        nc.gpsimd.dma_start(wv_sb[:], wv.rearrange("(kc p) m -> p kc m", p=128))
        nc.gpsimd.dma_start(wo_sb[:], wo.rearrange("(j p) n -> p j n", p=128))
        xt_r = xt.rearrange("(kc p) t -> p kc t", p=128)
        for kc in range(CCH):
            for t4 in range(NQB):
                nc.sync.dma_start(
                    xt_c[kc][:, t4 * 512 : (t4 + 1) * 512],
                    xt_r[:, kc, t4 * 512 : (t4 + 1) * 512],
                )

        make_identity(nc, ident_sb[:])
        # ones column of v' (fused softmax denominator)
        nc.gpsimd.memset(vv_sb[:, :, :, DH : DH + 1], 1.0)
        # shifted causal masks: keep where (tq_local) - (tk_local) - 128*m >= 0
        nc.gpsimd.memset(masks_sb[:], 1.0)
        for m in range(NQB):
            nc.gpsimd.affine_select(
                out=masks_sb[:, m, :],
                in_=masks_sb[:, m, :],
                compare_op=mybir.AluOpType.is_ge,
                fill=0.0,
                base=-(128 * m),
                channel_multiplier=-1,
                pattern=[[1, QB]],
            )

        # ---- phase 2: attention + per-block normalize + output projection ----
        with (
            tc.tile_pool(name="s_ps", bufs=2, space="PSUM") as s_psum,
            tc.tile_pool(name="o_ps", bufs=1, space="PSUM") as o_psum,
            tc.tile_pool(name="pt_pool", bufs=6) as pt_pool,
            tc.tile_pool(name="o_stage", bufs=4) as o_stage,
            tc.tile_pool(name="dram", bufs=1, space="DRAM") as dram_pool,
        ):
            scratch = dram_pool.tile([NH, T], f32)

            # HAM warmup: keep PE busy while input DMAs stream in so the
            # first real matmuls run at full clock
            warm_ps = s_psum.tile([128, 2 * QB], f32, tag="s", name="warm")
            for _ in range(30):
                nc.tensor.matmul(
                    warm_ps[:, 0:128], ident_sb[:], ident_sb[:, 0:128],
                    start=True, stop=True,
                )

            def proj_block(i):
                qps = s_psum.tile([128, 2 * QB], f32, tag="s", name=f"qp{i}")
                kps = s_psum.tile([128, 2 * QB], f32, tag="s", name=f"kp{i}")
                vps = [
                    o_psum.tile([128, 512], f32, tag=f"o{tt % 4}", name=f"vp{tt}")
                    for tt in range(4 * i, 4 * i + 4)
                ]
                for kc in range(CCH):
                    for ps2, w_sb in ((qps, wq_sb), (kps, wk_sb)):
                        for j in range(2):
                            nc.tensor.matmul(
                                ps2[:, j * QB : (j + 1) * QB],
                                w_sb[:, kc, j * 128 : (j + 1) * 128],
                                xt_c[kc][:, i * QB : (i + 1) * QB],
                                start=(kc == 0),
                                stop=(kc == CCH - 1),
                            )
                    for u, tt in enumerate(range(4 * i, 4 * i + 4)):
                        nc.tensor.matmul(
                            vps[u][:, 0 : NH * DH],
                            xt_c[kc][:, tt * 128 : (tt + 1) * 128],
                            wv_sb[:, kc, :],
                            start=(kc == 0),
                            stop=(kc == CCH - 1),
                        )
                for ps2, dst in ((qps, qt_sb), (kps, kt_sb)):
                    for j in range(2):
                        nc.vector.tensor_copy(
                            dst[:, j, i * QB : (i + 1) * QB],
                            ps2[:, j * QB : (j + 1) * QB],
                        )
                for u, tt in enumerate(range(4 * i, 4 * i + 4)):
                    nc.vector.tensor_copy(
                        vv_sb[:, tt, :, 0:DH],
                        vps[u][:, 0 : NH * DH].rearrange("p (h d) -> p h d", h=NH),
                    )

            def attention_tile(qb, kj, o_ps):
                nkj = 4 * (qb + 1)
                diag = kj >= 4 * qb
                m = kj - 4 * qb if diag else 0
                w0 = m * 128          # first valid local query column
                W = QB - w0           # valid width
                for pr in range(2):  # head pair (2*pr, 2*pr+1)
                    s2 = s_psum.tile(
                        [128, 2 * QB], f32, tag="s", name=f"s_{qb}_{kj}_{pr}"
                    )
                    for hh in range(2):
                        nc.tensor.matmul(
                            s2[:, hh * QB + w0 : (hh + 1) * QB],
                            kt_sb[64 * hh : 64 * hh + 64, pr, kj * 128 : (kj + 1) * 128],
                            qt_sb[64 * hh : 64 * hh + 64, pr, qb * QB + w0 : (qb + 1) * QB],
                            start=True,
                            stop=True,
                        )
                    pt = pt_pool.tile([128, 2 * QB], bf16, tag="pt")
                    s2v = s2[:].rearrange("p (hh q) -> p hh q", hh=2)
                    ptv = pt[:].rearrange("p (hh q) -> p hh q", hh=2)
                    nc.scalar.activation(
                        ptv[:, :, w0:QB],
                        s2v[:, :, w0:QB],
                        mybir.ActivationFunctionType.Exp,
                        scale=0.125,
                    )
                    if diag:
                        nc.vector.tensor_tensor(
                            ptv[:, :, w0:QB],
                            ptv[:, :, w0:QB],
                            masks_sb[:, m, None, w0:QB].to_broadcast((128, 2, W)),
                            mybir.AluOpType.mult,
                        )
                    for hh in range(2):
                        h = 2 * pr + hh
                        nc.tensor.matmul(
                            o_ps[h][:, w0:QB],
                            vv_sb[:, kj, h, :],
                            pt[:, hh * QB + w0 : (hh + 1) * QB],
                            start=(kj == 0),
                            stop=(kj == nkj - 1),
                        )

            def attention_tail(qb, o_ps):
                # rowsums first (they gate the normalize chain), on the scalar
                # engine so they run parallel to the DVE otu copies
                for h in range(NH):
                    nc.scalar.copy(
                        rs_sb[32 * h : 32 * h + 1, qb * QB : (qb + 1) * QB],
                        o_ps[h][DH : DH + 1, :],
                    )
                for h in range(NH):
                    bh = 64 * (h % 2)
                    chh = h // 2
                    nc.vector.tensor_copy(
                        otu_sb[bh : bh + 64, chh, qb * QB : (qb + 1) * QB],
                        o_ps[h][0:DH, :],
                    )

            def norm_block(qb):
                for tt in range(4 * qb, 4 * qb + 4):
                    pt_ = s_psum.tile([128, 512], f32, tag="s", name=f"t1_{tt}")
                    nc.tensor.transpose(
                        pt_[:, 0:128], rs_sb[:, tt * 128 : (tt + 1) * 128], ident_sb[:]
                    )
                    nc.vector.tensor_copy(rst_sb[:, tt, :], pt_[:, 0:128:32])
                nc.vector.reciprocal(
                    rrt_sb[:, 4 * qb : 4 * qb + 4, :], rst_sb[:, 4 * qb : 4 * qb + 4, :]
                )
                for tt in range(4 * qb, 4 * qb + 4):
                    pb = s_psum.tile([128, 512], f32, tag="s", name=f"t2_{tt}")
                    nc.tensor.transpose(pb[0:NH, 0:128], rrt_sb[:, tt, :], ident_sb[:])
                    nc.vector.tensor_copy(
                        rr4_sb[:, tt * 128 : (tt + 1) * 128], pb[0:NH, 0:128]
                    )
                nc.sync.dma_start(
                    scratch[:, qb * QB : (qb + 1) * QB],
                    rr4_sb[:, qb * QB : (qb + 1) * QB],
                )
                for h in range(NH):
                    bh = 64 * (h % 2)
                    chh = h // 2
                    src_ap = bass.AP(
                        scratch.tensor,
                        scratch.offset + h * T + qb * QB,
                        [[0, 64], [1, QB]],
                    )
                    eng = nc.sync if h % 2 == 0 else nc.gpsimd
                    eng.dma_start(
                        rb_sb[bh : bh + 64, chh, qb * QB : (qb + 1) * QB], src_ap
                    )
                for j in range(2):
                    nc.vector.tensor_tensor(
                        yt_sb[:, j, qb * QB : (qb + 1) * QB],
                        otu_sb[:, j, qb * QB : (qb + 1) * QB],
                        rb_sb[:, j, qb * QB : (qb + 1) * QB],
                        mybir.AluOpType.mult,
                    )

            def wo_block(qb, last=False):
                for tt in range(4 * qb, 4 * qb + 4):
                    for n in range(2):
                        ps = s_psum.tile([128, 512], f32, tag="s", name=f"wo_{tt}_{n}")
                        for j in range(2):
                            nc.tensor.matmul(
                                ps[:],
                                yt_sb[:, j, tt * 128 : (tt + 1) * 128],
                                wo_sb[:, j, n * 512 : (n + 1) * 512],
                                start=(j == 0),
                                stop=(j == 1),
                            )
                        ot = o_stage.tile([128, 512], f32, tag="ot")
                        if last and (tt + n) % 2 == 0:
                            nc.scalar.copy(ot[:], ps[:])
                        else:
                            nc.vector.tensor_copy(ot[:], ps[:])
                        eng = nc.sync if (tt + n) % 2 == 0 else nc.gpsimd
                        eng.dma_start(
                            out[tt * 128 : (tt + 1) * 128, n * 512 : (n + 1) * 512],
                            ot[:],
                        )

            for qb in range(NQB):
                proj_block(qb)
                o_ps = [
                    o_psum.tile([DH + 1, QB], f32, tag=f"o{h}", name=f"o_{qb}_{h}")
                    for h in range(NH)
                ]
                nkj = 4 * (qb + 1)
                for kj in range(nkj):
                    attention_tile(qb, kj, o_ps)
                    if qb >= 1 and kj == 1:
                        norm_block(qb - 1)
                    if qb >= 1 and kj == min(6, nkj - 2):
                        wo_block(qb - 1)
                attention_tail(qb, o_ps)
            norm_block(NQB - 1)
            wo_block(NQB - 1, last=True)
    return nc


_NC = None
LAST_RESULT = None


def kernel(x, Wq, Wk, Wv, Wo):
    global _NC, LAST_RESULT
    from concourse.bass_utils import run_bass_kernel_spmd

    if _NC is None:
        _NC = build_nc()

    x = np.asarray(x, dtype=np.float32)
    import ml_dtypes

    def b(a):
        return np.ascontiguousarray(a).astype(ml_dtypes.bfloat16)

    in_maps = []
    for core in range(8):
        bb = core // 4
        g = core % 4
        hs = slice(g * NH * DH, (g + 1) * NH * DH)
        in_maps.append(
            {
                "xt": b(x[bb].T),
                "wq": b(np.asarray(Wq)[hs, :].T),
                "wk": b(np.asarray(Wk)[hs, :].T),
                "wv": b(np.asarray(Wv)[hs, :].T),
                "wo": b(np.asarray(Wo)[:, hs].T),
            }
        )

    LAST_RESULT = run_bass_kernel_spmd(_NC, in_maps, core_ids=list(range(8)))
    res = LAST_RESULT.results
    out = np.zeros((2, T, C), dtype=np.float32)
    for core in range(8):
        out[core // 4] += res[core]["out"]
    return out


# revision 23
# speedup vs baseline: 1.0066x; 1.0066x over previous
"""Causal self-attention (B=2, T=2048, D=1024, H=16) on 8 trn2 NeuronCores.

Sharding: core c handles batch b=c//4 and head group g=c%4 (4 heads each).
Each core computes q/k/v projections for its heads, causal attention, and a
partial output projection against its slice of Wo; the host sums the 4
partials per batch.
"""

import os
import sys
import types

sys.path.insert(0, "/opt/trn_rl_repo")

import numpy as np
import orjson

import concourse.bass as bass
import concourse.mybir as mybir
import concourse.tile as tile
from concourse.masks import make_identity

f32 = mybir.dt.float32
bf16 = mybir.dt.bfloat16

T = 2048
C = 1024
NH = 4          # heads per core
DH = 64
QB = 512        # query block
NQB = T // QB   # 4
KT = 128        # key tile
NKT = T // KT   # 16
CCH = C // 128  # 8 contraction chunks


def _install_ntff_hook():
    """Provide antenv.axon_hooks (absent in this image) so trace=True /
    BASS_TRACE=1 profiling works under axon instead of crashing."""
    try:
        import antenv.axon_hooks  # noqa: F401
        return
    except ImportError:
        pass
    try:
        mod = types.ModuleType("antenv.axon_hooks")
        _h = [None]
        mod.set_axon_ntff_profile_hook = lambda h: _h.__setitem__(0, h)
        mod.get_axon_ntff_profile_hook = lambda: _h[0]
        sys.modules["antenv.axon_hooks"] = mod
        import antenv

        antenv.axon_hooks = mod
        from trn_agent_boot.trn_boot import _ntff_profile_via_ctypes

        so = "/opt/axon/libaxon_pjrt.so"
        if os.path.exists(so):
            hook = _ntff_profile_via_ctypes(so)
            if hook is not None:
                mod.set_axon_ntff_profile_hook(hook)
    except Exception:
        pass


_install_ntff_hook()



def _split_multi_waits(bir: dict) -> dict:
    """The walrus build here rejects instructions with >1 semaphore wait.
    Hoist extra waits onto EventSemaphore insts inserted just before the
    offending instruction on the same engine (semantically equivalent)."""
    for fn in bir.get("functions", []):
        for bb in fn.get("blocks", []):
            out = []
            changed = False
            for inst in bb.get("instructions", []):
                si = inst.get("sync_info")
                waits = si.get("on_wait") if si else None
                if waits and len(waits) > 1:
                    changed = True
                    for i, w in enumerate(waits[:-1]):
                        out.append(
                            {
                                "debug": inst.get("debug", 0),
                                "engine": inst["engine"],
                                "ins": [],
                                "name": f"{inst.get('name', 'I')}-hw{i}",
                                "opcode": "EventSemaphore",
                                "outs": [],
                                "sync_info": {"on_update": [], "on_wait": [w]},
                            }
                        )
                    si["on_wait"] = [waits[-1]]
                out.append(inst)
            if changed:
                bb["instructions"] = out
    return bir


class PatchedBass(bass.Bass):
    def to_json_bytes(self) -> bytes:
        raw = super().to_json_bytes()
        return orjson.dumps(_split_multi_waits(orjson.loads(raw)))


def build_nc():
    nc = PatchedBass(name="causal_attn")
    xt = nc.dram_tensor("xt", [C, T], bf16, kind="ExternalInput")
    wq = nc.dram_tensor("wq", [C, NH * DH], bf16, kind="ExternalInput")
    wk = nc.dram_tensor("wk", [C, NH * DH], bf16, kind="ExternalInput")
    wv = nc.dram_tensor("wv", [C, NH * DH], bf16, kind="ExternalInput")
    wo = nc.dram_tensor("wo", [NH * DH, C], bf16, kind="ExternalInput")
    out = nc.dram_tensor("out", [T, C], f32, kind="ExternalOutput")

    from contextlib import ExitStack

    with tile.TileContext(nc) as tc, ExitStack() as ctx:
        cp = ctx.enter_context(tc.tile_pool(name="const", bufs=1))
        # persistent SBUF tensors
        xt_c = [cp.tile([128, T], bf16, name=f"xt_c{kc}") for kc in range(CCH)]
        wq_sb = cp.tile([128, CCH, NH * DH], bf16)
        wk_sb = cp.tile([128, CCH, NH * DH], bf16)
        wv_sb = cp.tile([128, CCH, NH * DH], bf16)
        wo_sb = cp.tile([128, 2, C], bf16)
        qt_sb = cp.tile([128, 2, T], bf16)
        kt_sb = cp.tile([128, 2, T], bf16)
        vv_sb = cp.tile([128, NKT, NH, DH + 1], bf16)
        otu_sb = cp.tile([128, 2, T], f32)
        yt_sb = cp.tile([128, 2, T], bf16)
        masks_sb = cp.tile([128, NQB, QB], bf16)
        ident_sb = cp.tile([128, 128], f32)
        rs_sb = cp.tile([128, T], f32)
        rst_sb = cp.tile([128, NKT, NH], f32)
        rrt_sb = cp.tile([128, NKT, NH], f32)
        rr4_sb = cp.tile([NH, T], f32)
        rb_sb = cp.tile([128, 2, T], f32)

        # weights on the gpsimd queue, xt per-chunk on sync so projection
        # matmuls can start as soon as each chunk lands
        nc.sync.dma_start(wq_sb[:], wq.rearrange("(kc p) m -> p kc m", p=128))
        nc.gpsimd.dma_start(wk_sb[:], wk.rearrange("(kc p) m -> p kc m", p=128))
        nc.gpsimd.dma_start(wv_sb[:], wv.rearrange("(kc p) m -> p kc m", p=128))
        nc.gpsimd.dma_start(wo_sb[:], wo.rearrange("(j p) n -> p j n", p=128))
        xt_r = xt.rearrange("(kc p) t -> p kc t", p=128)
        for kc in range(CCH):
            for t4 in range(NQB):
                nc.sync.dma_start(
                    xt_c[kc][:, t4 * 512 : (t4 + 1) * 512],
                    xt_r[:, kc, t4 * 512 : (t4 + 1) * 512],
                )

        make_identity(nc, ident_sb[:])
        # ones column of v' (fused softmax denominator)
        nc.gpsimd.memset(vv_sb[:, :, :, DH : DH + 1], 1.0)
        # shifted causal masks: keep where (tq_local) - (tk_local) - 128*m >= 0
        nc.gpsimd.memset(masks_sb[:], 1.0)
        for m in range(NQB):
            nc.gpsimd.affine_select(
                out=masks_sb[:, m, :],
                in_=masks_sb[:, m, :],
                compare_op=mybir.AluOpType.is_ge,
                fill=0.0,
                base=-(128 * m),
                channel_multiplier=-1,
                pattern=[[1, QB]],
            )

        # ---- phase 2: attention + per-block normalize + output projection ----
        with (
            tc.tile_pool(name="s_ps", bufs=2, space="PSUM") as s_psum,
            tc.tile_pool(name="o_ps", bufs=1, space="PSUM") as o_psum,
            tc.tile_pool(name="pt_pool", bufs=6) as pt_pool,
            tc.tile_pool(name="o_stage", bufs=4) as o_stage,
            tc.tile_pool(name="dram", bufs=1, space="DRAM") as dram_pool,
        ):
            scratch = dram_pool.tile([NH, T], f32)

            def proj_block(i):
                qps = s_psum.tile([128, 2 * QB], f32, tag="s", name=f"qp{i}")
                kps = s_psum.tile([128, 2 * QB], f32, tag="s", name=f"kp{i}")
                vps = [
                    o_psum.tile([128, 512], f32, tag=f"o{tt % 4}", name=f"vp{tt}")
                    for tt in range(4 * i, 4 * i + 4)
                ]
                for kc in range(CCH):
                    for ps2, w_sb in ((qps, wq_sb), (kps, wk_sb)):
                        for j in range(2):
                            nc.tensor.matmul(
                                ps2[:, j * QB : (j + 1) * QB],
                                w_sb[:, kc, j * 128 : (j + 1) * 128],
                                xt_c[kc][:, i * QB : (i + 1) * QB],
                                start=(kc == 0),
                                stop=(kc == CCH - 1),
                            )
                    for u, tt in enumerate(range(4 * i, 4 * i + 4)):
                        nc.tensor.matmul(
                            vps[u][:, 0 : NH * DH],
                            xt_c[kc][:, tt * 128 : (tt + 1) * 128],
                            wv_sb[:, kc, :],
                            start=(kc == 0),
                            stop=(kc == CCH - 1),
                        )
                for ps2, dst in ((qps, qt_sb), (kps, kt_sb)):
                    for j in range(2):
                        nc.vector.tensor_copy(
                            dst[:, j, i * QB : (i + 1) * QB],
                            ps2[:, j * QB : (j + 1) * QB],
                        )
                for u, tt in enumerate(range(4 * i, 4 * i + 4)):
                    nc.vector.tensor_copy(
                        vv_sb[:, tt, :, 0:DH],
                        vps[u][:, 0 : NH * DH].rearrange("p (h d) -> p h d", h=NH),
                    )

            def attention_tile(qb, kj, o_ps):
                nkj = 4 * (qb + 1)
                diag = kj >= 4 * qb
                m = kj - 4 * qb if diag else 0
                w0 = m * 128          # first valid local query column
                W = QB - w0           # valid width
                for pr in range(2):  # head pair (2*pr, 2*pr+1)
                    s2 = s_psum.tile(
                        [128, 2 * QB], f32, tag="s", name=f"s_{qb}_{kj}_{pr}"
                    )
                    for hh in range(2):
                        nc.tensor.matmul(
                            s2[:, hh * QB + w0 : (hh + 1) * QB],
                            kt_sb[64 * hh : 64 * hh + 64, pr, kj * 128 : (kj + 1) * 128],
                            qt_sb[64 * hh : 64 * hh + 64, pr, qb * QB + w0 : (qb + 1) * QB],
                            start=True,
                            stop=True,
                        )
                    pt = pt_pool.tile([128, 2 * QB], bf16, tag="pt")
                    s2v = s2[:].rearrange("p (hh q) -> p hh q", hh=2)
                    ptv = pt[:].rearrange("p (hh q) -> p hh q", hh=2)
                    nc.scalar.activation(
                        ptv[:, :, w0:QB],
                        s2v[:, :, w0:QB],
                        mybir.ActivationFunctionType.Exp,
                        scale=0.125,
                    )
                    if diag:
                        nc.vector.tensor_tensor(
                            ptv[:, :, w0:QB],
                            ptv[:, :, w0:QB],
                            masks_sb[:, m, None, w0:QB].to_broadcast((128, 2, W)),
                            mybir.AluOpType.mult,
                        )
                    for hh in range(2):
                        h = 2 * pr + hh
                        nc.tensor.matmul(
                            o_ps[h][:, w0:QB],
                            vv_sb[:, kj, h, :],
                            pt[:, hh * QB + w0 : (hh + 1) * QB],
                            start=(kj == 0),
                            stop=(kj == nkj - 1),
                        )

            def attention_tail(qb, o_ps):
                # rowsums first (they gate the normalize chain), on the scalar
                # engine so they run parallel to the DVE otu copies
                for h in range(NH):
                    nc.scalar.copy(
                        rs_sb[32 * h : 32 * h + 1, qb * QB : (qb + 1) * QB],
                        o_ps[h][DH : DH + 1, :],
                    )
                for h in range(NH):
                    bh = 64 * (h % 2)
                    chh = h // 2
                    nc.vector.tensor_copy(
                        otu_sb[bh : bh + 64, chh, qb * QB : (qb + 1) * QB],
                        o_ps[h][0:DH, :],
                    )

            def norm_block(qb):
                for tt in range(4 * qb, 4 * qb + 4):
                    pt_ = s_psum.tile([128, 512], f32, tag="s", name=f"t1_{tt}")
                    nc.tensor.transpose(
                        pt_[:, 0:128], rs_sb[:, tt * 128 : (tt + 1) * 128], ident_sb[:]
                    )
                    nc.vector.tensor_copy(rst_sb[:, tt, :], pt_[:, 0:128:32])
                nc.vector.reciprocal(
                    rrt_sb[:, 4 * qb : 4 * qb + 4, :], rst_sb[:, 4 * qb : 4 * qb + 4, :]
                )
                for tt in range(4 * qb, 4 * qb + 4):
                    pb = s_psum.tile([128, 512], f32, tag="s", name=f"t2_{tt}")
                    nc.tensor.transpose(pb[0:NH, 0:128], rrt_sb[:, tt, :], ident_sb[:])
                    nc.vector.tensor_copy(
                        rr4_sb[:, tt * 128 : (tt + 1) * 128], pb[0:NH, 0:128]
                    )
                nc.sync.dma_start(
                    scratch[:, qb * QB : (qb + 1) * QB],
                    rr4_sb[:, qb * QB : (qb + 1) * QB],
                )
                for h in range(NH):
                    bh = 64 * (h % 2)
                    chh = h // 2
                    src_ap = bass.AP(
                        scratch.tensor,
                        scratch.offset + h * T + qb * QB,
                        [[0, 64], [1, QB]],
                    )
                    eng = nc.sync if h % 2 == 0 else nc.gpsimd
                    eng.dma_start(
                        rb_sb[bh : bh + 64, chh, qb * QB : (qb + 1) * QB], src_ap
                    )
                for j in range(2):
                    nc.vector.tensor_tensor(
                        yt_sb[:, j, qb * QB : (qb + 1) * QB],
                        otu_sb[:, j, qb * QB : (qb + 1) * QB],
                        rb_sb[:, j, qb * QB : (qb + 1) * QB],
                        mybir.AluOpType.mult,
                    )

            def wo_block(qb, last=False):
                for tt in range(4 * qb, 4 * qb + 4):
                    for n in range(2):
                        ps = s_psum.tile([128, 512], f32, tag="s", name=f"wo_{tt}_{n}")
                        for j in range(2):
                            nc.tensor.matmul(
                                ps[:],
                                yt_sb[:, j, tt * 128 : (tt + 1) * 128],
                                wo_sb[:, j, n * 512 : (n + 1) * 512],
                                start=(j == 0),
                                stop=(j == 1),
                            )
                        ot = o_stage.tile([128, 512], f32, tag="ot")
                        if last and (tt + n) % 2 == 0:
                            nc.scalar.copy(ot[:], ps[:])
                        else:
                            nc.vector.tensor_copy(ot[:], ps[:])
                        eng = nc.sync if (tt + n) % 2 == 0 else nc.gpsimd
                        eng.dma_start(
                            out[tt * 128 : (tt + 1) * 128, n * 512 : (n + 1) * 512],
                            ot[:],
                        )

            for qb in range(NQB):
                proj_block(qb)
                o_ps = [
                    o_psum.tile([DH + 1, QB], f32, tag=f"o{h}", name=f"o_{qb}_{h}")
                    for h in range(NH)
                ]
                nkj = 4 * (qb + 1)
                for kj in range(nkj):
                    attention_tile(qb, kj, o_ps)
                    if qb >= 1 and kj == 1:
                        norm_block(qb - 1)
                    if qb >= 1 and kj == min(6, nkj - 2):
                        wo_block(qb - 1)
                attention_tail(qb, o_ps)
            norm_block(NQB - 1)
            wo_block(NQB - 1, last=True)
    return nc


_NC = None
LAST_RESULT = None


def kernel(x, Wq, Wk, Wv, Wo):
    global _NC, LAST_RESULT
    from concourse.bass_utils import run_bass_kernel_spmd

    if _NC is None:
        _NC = build_nc()

    x = np.asarray(x, dtype=np.float32)
    import ml_dtypes

    def b(a):
        return np.ascontiguousarray(a).astype(ml_dtypes.bfloat16)

    in_maps = []
    for core in range(8):
        bb = core // 4
        g = core % 4
        hs = slice(g * NH * DH, (g + 1) * NH * DH)
        in_maps.append(
            {
                "xt": b(x[bb].T),
                "wq": b(np.asarray(Wq)[hs, :].T),
                "wk": b(np.asarray(Wk)[hs, :].T),
                "wv": b(np.asarray(Wv)[hs, :].T),
                "wo": b(np.asarray(Wo)[:, hs].T),
            }
        )

    LAST_RESULT = run_bass_kernel_spmd(_NC, in_maps, core_ids=list(range(8)))
    res = LAST_RESULT.results
    out = np.zeros((2, T, C), dtype=np.float32)
    for core in range(8):
        out[core // 4] += res[core]["out"]
    return out


# revision 24
# speedup vs baseline: 1.0072x; 1.0006x over previous
"""Causal self-attention (B=2, T=2048, D=1024, H=16) on 8 trn2 NeuronCores.

Sharding: core c handles batch b=c//4 and head group g=c%4 (4 heads each).
Each core computes q/k/v projections for its heads, causal attention, and a
partial output projection against its slice of Wo; the host sums the 4
partials per batch.
"""

import os
import sys
import types

sys.path.insert(0, "/opt/trn_rl_repo")

import numpy as np
import orjson

import concourse.bass as bass
import concourse.mybir as mybir
import concourse.tile as tile
from concourse.masks import make_identity

f32 = mybir.dt.float32
bf16 = mybir.dt.bfloat16

T = 2048
C = 1024
NH = 4          # heads per core
DH = 64
QB = 512        # query block
NQB = T // QB   # 4
KT = 128        # key tile
NKT = T // KT   # 16
CCH = C // 128  # 8 contraction chunks


def _install_ntff_hook():
    """Provide antenv.axon_hooks (absent in this image) so trace=True /
    BASS_TRACE=1 profiling works under axon instead of crashing."""
    try:
        import antenv.axon_hooks  # noqa: F401
        return
    except ImportError:
        pass
    try:
        mod = types.ModuleType("antenv.axon_hooks")
        _h = [None]
        mod.set_axon_ntff_profile_hook = lambda h: _h.__setitem__(0, h)
        mod.get_axon_ntff_profile_hook = lambda: _h[0]
        sys.modules["antenv.axon_hooks"] = mod
        import antenv

        antenv.axon_hooks = mod
        from trn_agent_boot.trn_boot import _ntff_profile_via_ctypes

        so = "/opt/axon/libaxon_pjrt.so"
        if os.path.exists(so):
            hook = _ntff_profile_via_ctypes(so)
            if hook is not None:
                mod.set_axon_ntff_profile_hook(hook)
    except Exception:
        pass


_install_ntff_hook()



def _split_multi_waits(bir: dict) -> dict:
    """The walrus build here rejects instructions with >1 semaphore wait.
    Hoist extra waits onto EventSemaphore insts inserted just before the
    offending instruction on the same engine (semantically equivalent)."""
    for fn in bir.get("functions", []):
        for bb in fn.get("blocks", []):
            out = []
            changed = False
            for inst in bb.get("instructions", []):
                si = inst.get("sync_info")
                waits = si.get("on_wait") if si else None
                if waits and len(waits) > 1:
                    changed = True
                    for i, w in enumerate(waits[:-1]):
                        out.append(
                            {
                                "debug": inst.get("debug", 0),
                                "engine": inst["engine"],
                                "ins": [],
                                "name": f"{inst.get('name', 'I')}-hw{i}",
                                "opcode": "EventSemaphore",
                                "outs": [],
                                "sync_info": {"on_update": [], "on_wait": [w]},
                            }
                        )
                    si["on_wait"] = [waits[-1]]
                out.append(inst)
            if changed:
                bb["instructions"] = out
    return bir


class PatchedBass(bass.Bass):
    def to_json_bytes(self) -> bytes:
        raw = super().to_json_bytes()
        return orjson.dumps(_split_multi_waits(orjson.loads(raw)))


def build_nc():
    nc = PatchedBass(name="causal_attn")
    xt = nc.dram_tensor("xt", [C, T], bf16, kind="ExternalInput")
    wq = nc.dram_tensor("wq", [C, NH * DH], bf16, kind="ExternalInput")
    wk = nc.dram_tensor("wk", [C, NH * DH], bf16, kind="ExternalInput")
    wv = nc.dram_tensor("wv", [C, NH * DH], bf16, kind="ExternalInput")
    wo = nc.dram_tensor("wo", [NH * DH, C], bf16, kind="ExternalInput")
    out = nc.dram_tensor("out", [T, C], f32, kind="ExternalOutput")

    from contextlib import ExitStack

    with tile.TileContext(nc) as tc, ExitStack() as ctx:
        cp = ctx.enter_context(tc.tile_pool(name="const", bufs=1))
        # persistent SBUF tensors
        xt_c = [cp.tile([128, T], bf16, name=f"xt_c{kc}") for kc in range(CCH)]
        wq_sb = cp.tile([128, CCH, NH * DH], bf16)
        wk_sb = cp.tile([128, CCH, NH * DH], bf16)
        wv_sb = cp.tile([128, CCH, NH * DH], bf16)
        wo_sb = cp.tile([128, 2, C], bf16)
        qt_sb = cp.tile([128, 2, T], bf16)
        kt_sb = cp.tile([128, 2, T], bf16)
        vv_sb = cp.tile([128, NKT, NH, DH + 1], bf16)
        otu_sb = cp.tile([128, 2, T], f32)
        yt_sb = cp.tile([128, 2, T], bf16)
        masks_sb = cp.tile([128, NQB, QB], bf16)
        ident_sb = cp.tile([128, 128], f32)
        rs_sb = cp.tile([128, T], f32)
        rst_sb = cp.tile([128, NKT, NH], f32)
        rrt_sb = cp.tile([128, NKT, NH], f32)
        rr4_sb = cp.tile([NH, T], f32)
        rb_sb = cp.tile([128, 2, T], f32)

        # weights on the gpsimd queue, xt per-chunk on sync so projection
        # matmuls can start as soon as each chunk lands
        nc.sync.dma_start(wq_sb[:], wq.rearrange("(kc p) m -> p kc m", p=128))
        nc.gpsimd.dma_start(wk_sb[:], wk.rearrange("(kc p) m -> p kc m", p=128))
        nc.gpsimd.dma_start(wv_sb[:], wv.rearrange("(kc p) m -> p kc m", p=128))
        nc.gpsimd.dma_start(wo_sb[:], wo.rearrange("(j p) n -> p j n", p=128))
        xt_r = xt.rearrange("(kc p) t -> p kc t", p=128)
        for kc in range(CCH):
            for t4 in range(NQB):
                nc.sync.dma_start(
                    xt_c[kc][:, t4 * 512 : (t4 + 1) * 512],
                    xt_r[:, kc, t4 * 512 : (t4 + 1) * 512],
                )

        make_identity(nc, ident_sb[:])
        # ones column of v' (fused softmax denominator)
        nc.gpsimd.memset(vv_sb[:, :, :, DH : DH + 1], 1.0)
        # shifted causal masks: keep where (tq_local) - (tk_local) - 128*m >= 0
        nc.gpsimd.memset(masks_sb[:], 1.0)
        for m in range(NQB):
            nc.gpsimd.affine_select(
                out=masks_sb[:, m, :],
                in_=masks_sb[:, m, :],
                compare_op=mybir.AluOpType.is_ge,
                fill=0.0,
                base=-(128 * m),
                channel_multiplier=-1,
                pattern=[[1, QB]],
            )

        # ---- phase 2: attention + per-block normalize + output projection ----
        with (
            tc.tile_pool(name="s_ps", bufs=2, space="PSUM") as s_psum,
            tc.tile_pool(name="o_ps", bufs=1, space="PSUM") as o_psum,
            tc.tile_pool(name="pt_pool", bufs=6) as pt_pool,
            tc.tile_pool(name="o_stage", bufs=4) as o_stage,
            tc.tile_pool(name="dram", bufs=1, space="DRAM") as dram_pool,
        ):
            scratch = dram_pool.tile([NH, T], f32)

            def proj_block(i):
                qps = s_psum.tile([128, 2 * QB], f32, tag="s", name=f"qp{i}")
                kps = s_psum.tile([128, 2 * QB], f32, tag="s", name=f"kp{i}")
                vps = [
                    o_psum.tile([128, 512], f32, tag=f"o{tt % 4}", name=f"vp{tt}")
                    for tt in range(4 * i, 4 * i + 4)
                ]
                for kc in range(CCH):
                    for ps2, w_sb in ((qps, wq_sb), (kps, wk_sb)):
                        for j in range(2):
                            nc.tensor.matmul(
                                ps2[:, j * QB : (j + 1) * QB],
                                w_sb[:, kc, j * 128 : (j + 1) * 128],
                                xt_c[kc][:, i * QB : (i + 1) * QB],
                                start=(kc == 0),
                                stop=(kc == CCH - 1),
                            )
                    for u, tt in enumerate(range(4 * i, 4 * i + 4)):
                        nc.tensor.matmul(
                            vps[u][:, 0 : NH * DH],
                            xt_c[kc][:, tt * 128 : (tt + 1) * 128],
                            wv_sb[:, kc, :],
                            start=(kc == 0),
                            stop=(kc == CCH - 1),
                        )
                for ps2, dst in ((qps, qt_sb), (kps, kt_sb)):
                    for j in range(2):
                        nc.vector.tensor_copy(
                            dst[:, j, i * QB : (i + 1) * QB],
                            ps2[:, j * QB : (j + 1) * QB],
                        )
                for u, tt in enumerate(range(4 * i, 4 * i + 4)):
                    nc.vector.tensor_copy(
                        vv_sb[:, tt, :, 0:DH],
                        vps[u][:, 0 : NH * DH].rearrange("p (h d) -> p h d", h=NH),
                    )

            def attention_tile(qb, kj, o_ps):
                nkj = 4 * (qb + 1)
                diag = kj >= 4 * qb
                m = kj - 4 * qb if diag else 0
                w0 = m * 128          # first valid local query column
                W = QB - w0           # valid width
                for pr in range(2):  # head pair (2*pr, 2*pr+1)
                    s2 = s_psum.tile(
                        [128, 2 * QB], f32, tag="s", name=f"s_{qb}_{kj}_{pr}"
                    )
                    for hh in range(2):
                        nc.tensor.matmul(
                            s2[:, hh * QB + w0 : (hh + 1) * QB],
                            kt_sb[64 * hh : 64 * hh + 64, pr, kj * 128 : (kj + 1) * 128],
                            qt_sb[64 * hh : 64 * hh + 64, pr, qb * QB + w0 : (qb + 1) * QB],
                            start=True,
                            stop=True,
                        )
                    pt = pt_pool.tile([128, 2 * QB], bf16, tag="pt")
                    s2v = s2[:].rearrange("p (hh q) -> p hh q", hh=2)
                    ptv = pt[:].rearrange("p (hh q) -> p hh q", hh=2)
                    nc.scalar.activation(
                        ptv[:, :, w0:QB],
                        s2v[:, :, w0:QB],
                        mybir.ActivationFunctionType.Exp,
                        scale=0.125,
                    )
                    if diag:
                        nc.vector.tensor_tensor(
                            ptv[:, :, w0:QB],
                            ptv[:, :, w0:QB],
                            masks_sb[:, m, None, w0:QB].to_broadcast((128, 2, W)),
                            mybir.AluOpType.mult,
                        )
                    for hh in range(2):
                        h = 2 * pr + hh
                        nc.tensor.matmul(
                            o_ps[h][:, w0:QB],
                            vv_sb[:, kj, h, :],
                            pt[:, hh * QB + w0 : (hh + 1) * QB],
                            start=(kj == 0),
                            stop=(kj == nkj - 1),
                        )

            def attention_tail(qb, o_ps):
                # rowsums first (they gate the normalize chain), on the scalar
                # engine so they run parallel to the DVE otu copies
                for h in range(NH):
                    nc.scalar.copy(
                        rs_sb[32 * h : 32 * h + 1, qb * QB : (qb + 1) * QB],
                        o_ps[h][DH : DH + 1, :],
                    )
                for h in range(NH):
                    bh = 64 * (h % 2)
                    chh = h // 2
                    nc.vector.tensor_copy(
                        otu_sb[bh : bh + 64, chh, qb * QB : (qb + 1) * QB],
                        o_ps[h][0:DH, :],
                    )

            def norm_block(qb):
                for tt in range(4 * qb, 4 * qb + 4):
                    pt_ = s_psum.tile([128, 512], f32, tag="s", name=f"t1_{tt}")
                    nc.tensor.transpose(
                        pt_[:, 0:128], rs_sb[:, tt * 128 : (tt + 1) * 128], ident_sb[:]
                    )
                    nc.vector.tensor_copy(rst_sb[:, tt, :], pt_[:, 0:128:32])
                nc.vector.reciprocal(
                    rrt_sb[:, 4 * qb : 4 * qb + 4, :], rst_sb[:, 4 * qb : 4 * qb + 4, :]
                )
                for tt in range(4 * qb, 4 * qb + 4):
                    pb = s_psum.tile([128, 512], f32, tag="s", name=f"t2_{tt}")
                    nc.tensor.transpose(pb[0:NH, 0:128], rrt_sb[:, tt, :], ident_sb[:])
                    nc.vector.tensor_copy(
                        rr4_sb[:, tt * 128 : (tt + 1) * 128], pb[0:NH, 0:128]
                    )
                nc.sync.dma_start(
                    scratch[:, qb * QB : (qb + 1) * QB],
                    rr4_sb[:, qb * QB : (qb + 1) * QB],
                )
                for h in range(NH):
                    bh = 64 * (h % 2)
                    chh = h // 2
                    src_ap = bass.AP(
                        scratch.tensor,
                        scratch.offset + h * T + qb * QB,
                        [[0, 64], [1, QB]],
                    )
                    eng = nc.sync if h % 2 == 0 else nc.gpsimd
                    eng.dma_start(
                        rb_sb[bh : bh + 64, chh, qb * QB : (qb + 1) * QB], src_ap
                    )
                for j in range(2):
                    nc.vector.tensor_tensor(
                        yt_sb[:, j, qb * QB : (qb + 1) * QB],
                        otu_sb[:, j, qb * QB : (qb + 1) * QB],
                        rb_sb[:, j, qb * QB : (qb + 1) * QB],
                        mybir.AluOpType.mult,
                    )

            def wo_block(qb, last=False):
                for tt in range(4 * qb, 4 * qb + 4):
                    for n in range(2):
                        ps = s_psum.tile([128, 512], f32, tag="s", name=f"wo_{tt}_{n}")
                        for j in range(2):
                            nc.tensor.matmul(
                                ps[:],
                                yt_sb[:, j, tt * 128 : (tt + 1) * 128],
                                wo_sb[:, j, n * 512 : (n + 1) * 512],
                                start=(j == 0),
                                stop=(j == 1),
                            )
                        ot = o_stage.tile([128, 512], f32, tag="ot")
                        nc.vector.tensor_copy(ot[:], ps[:])
                        eng = nc.sync if (tt + n) % 2 == 0 else nc.gpsimd
                        eng.dma_start(
                            out[tt * 128 : (tt + 1) * 128, n * 512 : (n + 1) * 512],
                            ot[:],
                        )

            for qb in range(NQB):
                proj_block(qb)
                o_ps = [
                    o_psum.tile([DH + 1, QB], f32, tag=f"o{h}", name=f"o_{qb}_{h}")
                    for h in range(NH)
                ]
                nkj = 4 * (qb + 1)
                for kj in range(nkj):
                    attention_tile(qb, kj, o_ps)
                    if qb >= 1 and kj == 1:
                        norm_block(qb - 1)
                    if qb >= 1 and kj == min(6, nkj - 2):
                        wo_block(qb - 1)
                attention_tail(qb, o_ps)
            norm_block(NQB - 1)
            wo_block(NQB - 1, last=True)
    return nc


_NC = None
LAST_RESULT = None


def kernel(x, Wq, Wk, Wv, Wo):
    global _NC, LAST_RESULT
    from concourse.bass_utils import run_bass_kernel_spmd

    if _NC is None:
        _NC = build_nc()

    x = np.asarray(x, dtype=np.float32)
    import ml_dtypes

    def b(a):
        return np.ascontiguousarray(a).astype(ml_dtypes.bfloat16)

    in_maps = []
    for core in range(8):
        bb = core // 4
        g = core % 4
        hs = slice(g * NH * DH, (g + 1) * NH * DH)
        in_maps.append(
            {
                "xt": b(x[bb].T),
                "wq": b(np.asarray(Wq)[hs, :].T),
                "wk": b(np.asarray(Wk)[hs, :].T),
                "wv": b(np.asarray(Wv)[hs, :].T),
                "wo": b(np.asarray(Wo)[:, hs].T),
            }
        )

    LAST_RESULT = run_bass_kernel_spmd(_NC, in_maps, core_ids=list(range(8)))
    res = LAST_RESULT.results
    out = np.zeros((2, T, C), dtype=np.float32)
    for core in range(8):
        out[core // 4] += res[core]["out"]
    return out


# revision 25
# speedup vs baseline: 1.0369x; 1.0295x over previous
"""Causal self-attention (B=2, T=2048, D=1024, H=16) on 8 trn2 NeuronCores.

Sharding: core c handles batch b=c//4 and head group g=c%4 (4 heads each).
Each core computes q/k/v projections for its heads, causal attention, and a
partial output projection against its slice of Wo; the host sums the 4
partials per batch.
"""

import os
import sys
import types

sys.path.insert(0, "/opt/trn_rl_repo")

import numpy as np
import orjson

import concourse.bass as bass
import concourse.mybir as mybir
import concourse.tile as tile
from concourse.masks import make_identity

f32 = mybir.dt.float32
bf16 = mybir.dt.bfloat16

T = 2048
C = 1024
NH = 4          # heads per core
DH = 64
QB = 512        # query block
NQB = T // QB   # 4
KT = 128        # key tile
NKT = T // KT   # 16
CCH = C // 128  # 8 contraction chunks


def _install_ntff_hook():
    """Provide antenv.axon_hooks (absent in this image) so trace=True /
    BASS_TRACE=1 profiling works under axon instead of crashing."""
    try:
        import antenv.axon_hooks  # noqa: F401
        return
    except ImportError:
        pass
    try:
        mod = types.ModuleType("antenv.axon_hooks")
        _h = [None]
        mod.set_axon_ntff_profile_hook = lambda h: _h.__setitem__(0, h)
        mod.get_axon_ntff_profile_hook = lambda: _h[0]
        sys.modules["antenv.axon_hooks"] = mod
        import antenv

        antenv.axon_hooks = mod
        from trn_agent_boot.trn_boot import _ntff_profile_via_ctypes

        so = "/opt/axon/libaxon_pjrt.so"
        if os.path.exists(so):
            hook = _ntff_profile_via_ctypes(so)
            if hook is not None:
                mod.set_axon_ntff_profile_hook(hook)
    except Exception:
        pass


_install_ntff_hook()



def _split_multi_waits(bir: dict) -> dict:
    """The walrus build here rejects instructions with >1 semaphore wait.
    Hoist extra waits onto EventSemaphore insts inserted just before the
    offending instruction on the same engine (semantically equivalent)."""
    for fn in bir.get("functions", []):
        for bb in fn.get("blocks", []):
            out = []
            changed = False
            for inst in bb.get("instructions", []):
                si = inst.get("sync_info")
                waits = si.get("on_wait") if si else None
                if waits and len(waits) > 1:
                    changed = True
                    for i, w in enumerate(waits[:-1]):
                        out.append(
                            {
                                "debug": inst.get("debug", 0),
                                "engine": inst["engine"],
                                "ins": [],
                                "name": f"{inst.get('name', 'I')}-hw{i}",
                                "opcode": "EventSemaphore",
                                "outs": [],
                                "sync_info": {"on_update": [], "on_wait": [w]},
                            }
                        )
                    si["on_wait"] = [waits[-1]]
                out.append(inst)
            if changed:
                bb["instructions"] = out
    return bir


class PatchedBass(bass.Bass):
    def to_json_bytes(self) -> bytes:
        raw = super().to_json_bytes()
        return orjson.dumps(_split_multi_waits(orjson.loads(raw)))


def build_nc():
    nc = PatchedBass(name="causal_attn")
    xt = nc.dram_tensor("xt", [C, T], bf16, kind="ExternalInput")
    wq = nc.dram_tensor("wq", [C, NH * DH], bf16, kind="ExternalInput")
    wk = nc.dram_tensor("wk", [C, NH * DH], bf16, kind="ExternalInput")
    wv = nc.dram_tensor("wv", [C, NH * DH], bf16, kind="ExternalInput")
    wo = nc.dram_tensor("wo", [NH * DH, C], bf16, kind="ExternalInput")
    out = nc.dram_tensor("out", [T, C], f32, kind="ExternalOutput")

    from contextlib import ExitStack

    with tile.TileContext(nc) as tc, ExitStack() as ctx:
        cp = ctx.enter_context(tc.tile_pool(name="const", bufs=1))
        # persistent SBUF tensors
        xt_c = [cp.tile([128, T], bf16, name=f"xt_c{kc}") for kc in range(CCH)]
        wq_sb = cp.tile([128, CCH, NH * DH], bf16)
        wk_sb = cp.tile([128, CCH, NH * DH], bf16)
        wv_sb = cp.tile([128, CCH, NH * DH], bf16)
        wo_sb = cp.tile([128, 2, C], bf16)
        qt_sb = cp.tile([128, 2, T], bf16)
        kt_sb = cp.tile([128, 2, T], bf16)
        vv_sb = cp.tile([128, NKT, NH, DH + 1], bf16)
        otu_sb = cp.tile([128, 2, T], f32)
        yt_sb = cp.tile([128, 2, T], bf16)
        mask_sb = cp.tile([128, 128], bf16)
        ident_sb = cp.tile([128, 128], f32)
        rs_sb = cp.tile([128, T], f32)
        rst_sb = cp.tile([128, NKT, NH], f32)
        rrt_sb = cp.tile([128, NKT, NH], f32)
        rr4_sb = cp.tile([NH, T], f32)
        rb_sb = cp.tile([128, 2, T], f32)

        # weights on the gpsimd queue, xt per-chunk on sync so projection
        # matmuls can start as soon as each chunk lands
        wq_r = wq.rearrange("(kc p) m -> p kc m", p=128)
        wk_r = wk.rearrange("(kc p) m -> p kc m", p=128)
        wv_r = wv.rearrange("(kc p) m -> p kc m", p=128)
        for kc in range(CCH):
            nc.sync.dma_start(wq_sb[:, kc, :], wq_r[:, kc, :])
            nc.gpsimd.dma_start(wk_sb[:, kc, :], wk_r[:, kc, :])
            nc.gpsimd.dma_start(wv_sb[:, kc, :], wv_r[:, kc, :])
        nc.gpsimd.dma_start(wo_sb[:], wo.rearrange("(j p) n -> p j n", p=128))
        xt_r = xt.rearrange("(kc p) t -> p kc t", p=128)
        for kc in range(CCH):
            for t4 in range(NQB):
                nc.sync.dma_start(
                    xt_c[kc][:, t4 * 512 : (t4 + 1) * 512],
                    xt_r[:, kc, t4 * 512 : (t4 + 1) * 512],
                )

        make_identity(nc, ident_sb[:])
        # ones column of v' (fused softmax denominator)
        nc.gpsimd.memset(vv_sb[:, :, :, DH : DH + 1], 1.0)
        # triangular mask: keep where tq_local - tk_local >= 0
        nc.gpsimd.memset(mask_sb[:], 1.0)
        nc.gpsimd.affine_select(
            out=mask_sb[:],
            in_=mask_sb[:],
            compare_op=mybir.AluOpType.is_ge,
            fill=0.0,
            base=0,
            channel_multiplier=-1,
            pattern=[[1, 128]],
        )

        # ---- phase 2: attention + per-block normalize + output projection ----
        with (
            tc.tile_pool(name="s_ps", bufs=2, space="PSUM") as s_psum,
            tc.tile_pool(name="o_ps", bufs=1, space="PSUM") as o_psum,
            tc.tile_pool(name="pt_pool", bufs=6) as pt_pool,
            tc.tile_pool(name="o_stage", bufs=4) as o_stage,
            tc.tile_pool(name="dram", bufs=1, space="DRAM") as dram_pool,
        ):
            scratch = dram_pool.tile([NH, T], f32)

            def proj_block(i):
                qps = s_psum.tile([128, 2 * QB], f32, tag="s", name=f"qp{i}")
                kps = s_psum.tile([128, 2 * QB], f32, tag="s", name=f"kp{i}")
                vps = [
                    o_psum.tile([128, 512], f32, tag=f"o{tt % 4}", name=f"vp{tt}")
                    for tt in range(4 * i, 4 * i + 4)
                ]
                for kc in range(CCH):
                    for ps2, w_sb in ((qps, wq_sb), (kps, wk_sb)):
                        for j in range(2):
                            nc.tensor.matmul(
                                ps2[:, j * QB : (j + 1) * QB],
                                w_sb[:, kc, j * 128 : (j + 1) * 128],
                                xt_c[kc][:, i * QB : (i + 1) * QB],
                                start=(kc == 0),
                                stop=(kc == CCH - 1),
                            )
                    for u, tt in enumerate(range(4 * i, 4 * i + 4)):
                        nc.tensor.matmul(
                            vps[u][:, 0 : NH * DH],
                            xt_c[kc][:, tt * 128 : (tt + 1) * 128],
                            wv_sb[:, kc, :],
                            start=(kc == 0),
                            stop=(kc == CCH - 1),
                        )
                for ps2, dst in ((qps, qt_sb), (kps, kt_sb)):
                    for j in range(2):
                        nc.vector.tensor_copy(
                            dst[:, j, i * QB : (i + 1) * QB],
                            ps2[:, j * QB : (j + 1) * QB],
                        )
                for u, tt in enumerate(range(4 * i, 4 * i + 4)):
                    nc.vector.tensor_copy(
                        vv_sb[:, tt, :, 0:DH],
                        vps[u][:, 0 : NH * DH].rearrange("p (h d) -> p h d", h=NH),
                    )

            def attention_tile(qb, kj, o_ps):
                nkj = 4 * (qb + 1)
                diag = kj >= 4 * qb
                m = kj - 4 * qb if diag else 0
                w0 = m * 128          # first valid local query column
                W = QB - w0           # valid width
                for pr in range(2):  # head pair (2*pr, 2*pr+1)
                    s2 = s_psum.tile(
                        [128, 2 * QB], f32, tag="s", name=f"s_{qb}_{kj}_{pr}"
                    )
                    for hh in range(2):
                        nc.tensor.matmul(
                            s2[:, hh * QB + w0 : (hh + 1) * QB],
                            kt_sb[64 * hh : 64 * hh + 64, pr, kj * 128 : (kj + 1) * 128],
                            qt_sb[64 * hh : 64 * hh + 64, pr, qb * QB + w0 : (qb + 1) * QB],
                            start=True,
                            stop=True,
                        )
                    pt = pt_pool.tile([128, 2 * QB], bf16, tag="pt")
                    s2v = s2[:].rearrange("p (hh q) -> p hh q", hh=2)
                    ptv = pt[:].rearrange("p (hh q) -> p hh q", hh=2)
                    nc.scalar.activation(
                        ptv[:, :, w0:QB],
                        s2v[:, :, w0:QB],
                        mybir.ActivationFunctionType.Exp,
                        scale=0.125,
                    )
                    if diag:
                        nc.vector.tensor_tensor(
                            ptv[:, :, w0 : w0 + 128],
                            ptv[:, :, w0 : w0 + 128],
                            mask_sb[:, None, :].to_broadcast((128, 2, 128)),
                            mybir.AluOpType.mult,
                        )
                    for hh in range(2):
                        h = 2 * pr + hh
                        nc.tensor.matmul(
                            o_ps[h][:, w0:QB],
                            vv_sb[:, kj, h, :],
                            pt[:, hh * QB + w0 : (hh + 1) * QB],
                            start=(kj == 0),
                            stop=(kj == nkj - 1),
                        )

            def attention_tail(qb, o_ps):
                # rowsums first (they gate the normalize chain), on the scalar
                # engine so they run parallel to the DVE otu copies
                for h in range(NH):
                    nc.scalar.copy(
                        rs_sb[32 * h : 32 * h + 1, qb * QB : (qb + 1) * QB],
                        o_ps[h][DH : DH + 1, :],
                    )
                for h in range(NH):
                    bh = 64 * (h % 2)
                    chh = h // 2
                    nc.vector.tensor_copy(
                        otu_sb[bh : bh + 64, chh, qb * QB : (qb + 1) * QB],
                        o_ps[h][0:DH, :],
                    )

            def norm_block(qb):
                for tt in range(4 * qb, 4 * qb + 4):
                    pt_ = s_psum.tile([128, 512], f32, tag="s", name=f"t1_{tt}")
                    nc.tensor.transpose(
                        pt_[:, 0:128], rs_sb[:, tt * 128 : (tt + 1) * 128], ident_sb[:]
                    )
                    nc.vector.tensor_copy(rst_sb[:, tt, :], pt_[:, 0:128:32])
                nc.vector.reciprocal(
                    rrt_sb[:, 4 * qb : 4 * qb + 4, :], rst_sb[:, 4 * qb : 4 * qb + 4, :]
                )
                for tt in range(4 * qb, 4 * qb + 4):
                    pb = s_psum.tile([128, 512], f32, tag="s", name=f"t2_{tt}")
                    nc.tensor.transpose(pb[0:NH, 0:128], rrt_sb[:, tt, :], ident_sb[:])
                    nc.vector.tensor_copy(
                        rr4_sb[:, tt * 128 : (tt + 1) * 128], pb[0:NH, 0:128]
                    )
                nc.sync.dma_start(
                    scratch[:, qb * QB : (qb + 1) * QB],
                    rr4_sb[:, qb * QB : (qb + 1) * QB],
                )
                for h in range(NH):
                    bh = 64 * (h % 2)
                    chh = h // 2
                    src_ap = bass.AP(
                        scratch.tensor,
                        scratch.offset + h * T + qb * QB,
                        [[0, 64], [1, QB]],
                    )
                    eng = nc.sync if h % 2 == 0 else nc.gpsimd
                    eng.dma_start(
                        rb_sb[bh : bh + 64, chh, qb * QB : (qb + 1) * QB], src_ap
                    )
                for j in range(2):
                    nc.vector.tensor_tensor(
                        yt_sb[:, j, qb * QB : (qb + 1) * QB],
                        otu_sb[:, j, qb * QB : (qb + 1) * QB],
                        rb_sb[:, j, qb * QB : (qb + 1) * QB],
                        mybir.AluOpType.mult,
                    )

            def wo_block(qb, last=False):
                for tt in range(4 * qb, 4 * qb + 4):
                    for n in range(2):
                        ps = s_psum.tile([128, 512], f32, tag="s", name=f"wo_{tt}_{n}")
                        for j in range(2):
                            nc.tensor.matmul(
                                ps[:],
                                yt_sb[:, j, tt * 128 : (tt + 1) * 128],
                                wo_sb[:, j, n * 512 : (n + 1) * 512],
                                start=(j == 0),
                                stop=(j == 1),
                            )
                        ot = o_stage.tile([128, 512], f32, tag="ot")
                        nc.vector.tensor_copy(ot[:], ps[:])
                        eng = nc.sync if (tt + n) % 2 == 0 else nc.gpsimd
                        eng.dma_start(
                            out[tt * 128 : (tt + 1) * 128, n * 512 : (n + 1) * 512],
                            ot[:],
                        )

            for qb in range(NQB):
                proj_block(qb)
                o_ps = [
                    o_psum.tile([DH + 1, QB], f32, tag=f"o{h}", name=f"o_{qb}_{h}")
                    for h in range(NH)
                ]
                nkj = 4 * (qb + 1)
                for kj in range(nkj):
                    attention_tile(qb, kj, o_ps)
                    if qb >= 1 and kj == 1:
                        norm_block(qb - 1)
                    if qb >= 1 and kj == min(6, nkj - 2):
                        wo_block(qb - 1)
                attention_tail(qb, o_ps)
            norm_block(NQB - 1)
            wo_block(NQB - 1, last=True)
    return nc


_NC = None
LAST_RESULT = None


def kernel(x, Wq, Wk, Wv, Wo):
    global _NC, LAST_RESULT
    from concourse.bass_utils import run_bass_kernel_spmd

    if _NC is None:
        _NC = build_nc()

    x = np.asarray(x, dtype=np.float32)
    import ml_dtypes

    def b(a):
        return np.ascontiguousarray(a).astype(ml_dtypes.bfloat16)

    in_maps = []
    for core in range(8):
        bb = core // 4
        g = core % 4
        hs = slice(g * NH * DH, (g + 1) * NH * DH)
        in_maps.append(
            {
                "xt": b(x[bb].T),
                "wq": b(np.asarray(Wq)[hs, :].T),
                "wk": b(np.asarray(Wk)[hs, :].T),
                "wv": b(np.asarray(Wv)[hs, :].T),
                "wo": b(np.asarray(Wo)[:, hs].T),
            }
        )

    LAST_RESULT = run_bass_kernel_spmd(_NC, in_maps, core_ids=list(range(8)))
    res = LAST_RESULT.results
    out = np.zeros((2, T, C), dtype=np.float32)
    for core in range(8):
        out[core // 4] += res[core]["out"]
    return out
